# revision 1
# baseline (speedup 1.0000x reference)
"""Trainium2 Bass kernel for nn_Attention (LayerNorm + MHA + rel-pos-bias + out proj).

Sharding: 16 heads / 8 cores = 2 consecutive heads per core (tensor parallel);
every core processes all 4 batches. Each core computes the partial
out-projection for its 2 heads; the host sums the 8 partials and adds b_out.

Device-side math (per core, heads h0=2c, h1=2c+1):
  xn   = (x - mu) * rsqrt(var + eps)            (ln_gamma folded into w_qkv,
                                                 ln_beta folded into qkv bias)
  Q^T  = (gamma*Wq_slice)^T xn^T  * 1/8         [128=2*64 qdims, tok]
  K^T  = (gamma*Wk_slice)^T xn^T                [128, tok]
  V    = xn Wv_slice                            [tok, 128]  (+ ones col -> V')
  S^T  = K_h Q_h^T                              [k, q] per head; both heads'
                                                 matmuls are row-group packed
  P^T  = exp(S^T) * exp(bias_h)^T               (bias passed pre-exp'd,
                                                 pre-transposed, bf16)
  O'^T = V'_h^T P^T                             [65, q]: rows 0-63 = O^T,
                                                 row 64 = softmax denominators
  OT   = O^T / denom
  Y    = O_2h W_out[2h rows]                    partial, bf16 out

All matmul operands are bf16 (fp32 matmul runs at 1/4 rate); accumulation is
fp32 in PSUM. Phases are emitted interleaved (A0 A1 [B0 C0] A2 [B1 C1] A3
[B2 C2] [B3 C3]) over one set of pools so Tile can pipeline them; PSUM is
budgeted statically at 8 banks: psS 2x[128,2,512] (phase A reuses the same
slots for QKV accumulation), psO 2x[65,512], psY 2x[128,512].
"""

import os
import sys

for _p in ("/opt/trn_rl_repo",):
    if os.path.isdir(_p) and _p not in sys.path:
        sys.path.insert(0, _p)

import numpy as np
import ml_dtypes

import bass_rust
import concourse.bass as bass
import concourse.mybir as mybir
import concourse.tile as tile
from concourse.bass_utils import run_bass_kernel_spmd

BF16 = mybir.dt.bfloat16
F32 = mybir.dt.float32
NPBF16 = ml_dtypes.bfloat16
AF = mybir.ActivationFunctionType
ALU = mybir.AluOpType

B, N, D = 4, 2048, 1024
HEADS, HD = 16, 64
P = 128
NCORES = 8
HPC = HEADS // NCORES          # heads per core = 2
TOK = B * N                    # 8192
QB = 512                       # q block for attention phase
NQB = N // QB                  # 4
NKT = N // P                   # 16 key tiles
DC = D // P                    # 8 model-dim chunks
GRP = 512                      # token group for QKV matmuls
NGRP = N // GRP                # 4 groups per batch
EPS = 1e-5
SCALE = HD ** -0.5


def _split_waits(nc, maxw=1):
    """This walrus build rejects instructions with more than one sync wait;
    move excess waits onto preceding same-engine NoOps (1 wait each)."""
    n_new = 0
    for bb in nc.main_func.blocks:
        out, changed = [], False
        for ins in bb.instructions:
            si = ins.sync_info
            if si is not None and si.on_wait and len(si.on_wait) > maxw:
                ow = list(si.on_wait)
                head, tail = ow[:-maxw], ow[-maxw:]
                for i, w in enumerate(head):
                    nop = mybir.InstNoOp(name=f"waitsplit_{ins.name}_{i}")
                    nop.engine = ins.engine
                    nop.sync_info = bass_rust.SyncInfo(on_wait=[w], on_update=[])
                    out.append(nop)
                    n_new += 1
                si.on_wait = tail
                changed = True
            out.append(ins)
        if changed:
            bb.instructions = out
    return n_new


def _build_graph(use_qkv_bias):
    nc = bass.Bass(target_bir_lowering=False)

    x = nc.declare_dram_parameter("x", [TOK, D], BF16, isOutput=False)
    wq = nc.declare_dram_parameter("wq", [D, P], BF16, isOutput=False)
    wk = nc.declare_dram_parameter("wk", [D, P], BF16, isOutput=False)
    wv = nc.declare_dram_parameter("wv", [D, P], BF16, isOutput=False)
    wo = nc.declare_dram_parameter("wo", [P, D], BF16, isOutput=False)
    ebt = nc.declare_dram_parameter("ebt", [HPC, NKT, NQB, P, QB], BF16, isOutput=False)
    if use_qkv_bias:
        qbq = nc.declare_dram_parameter("qbq", [P], F32, isOutput=False)
        qbk = nc.declare_dram_parameter("qbk", [P], F32, isOutput=False)
        qbv = nc.declare_dram_parameter("qbv", [P], F32, isOutput=False)
    y = nc.declare_dram_parameter("out", [TOK, D], BF16, isOutput=True)

    xn_dram = nc.dram_tensor("xn_scratch", [TOK, D], BF16)
    sums_dram = nc.dram_tensor("sums_scratch", [B, NQB, HPC, QB], F32)
    rec_dram = nc.dram_tensor("rec_scratch", [B, NQB, HPC, QB], F32)

    with tile.TileContext(nc) as tc:
        with tc.tile_pool(name="singles", bufs=1) as singles, \
             tc.tile_pool(name="pa_x", bufs=4) as pa_x, \
             tc.tile_pool(name="pa_xn", bufs=4) as pa_xn, \
             tc.tile_pool(name="pa_small", bufs=6) as pas, \
             tc.tile_pool(name="pa_xnt", bufs=1) as paxnt, \
             tc.tile_pool(name="pb_ebt", bufs=20) as pbe, \
             tc.tile_pool(name="pb_p", bufs=4) as pbp, \
             tc.tile_pool(name="pb_small", bufs=4) as pbs, \
             tc.tile_pool(name="pc_y", bufs=3) as pcy, \
             tc.tile_pool(name="psS", bufs=2, space="PSUM") as psS, \
             tc.tile_pool(name="psOY", bufs=4, space="PSUM") as psOY:

            # ---- persistent SBUF state ----
            wq_sb = singles.tile([P, DC, P], BF16, tag="wq")
            nc.sync.dma_start(wq_sb[:], wq.ap().rearrange("(c p) m -> p c m", p=P))
            wk_sb = singles.tile([P, DC, P], BF16, tag="wk")
            nc.sync.dma_start(wk_sb[:], wk.ap().rearrange("(c p) m -> p c m", p=P))
            wv_sb = singles.tile([P, DC, P], BF16, tag="wv")
            nc.sync.dma_start(wv_sb[:], wv.ap().rearrange("(c p) m -> p c m", p=P))
            wo_sb = singles.tile([P, D], BF16, tag="wo")
            nc.sync.dma_start(wo_sb[:], wo.ap())

            eps_sb = singles.tile([P, 1], F32, tag="eps")
            nc.vector.memset(eps_sb[:], EPS)

            if use_qkv_bias:
                qbq_sb = singles.tile([P, 1], F32, tag="qbq")
                nc.sync.dma_start(qbq_sb[:], qbq.ap()[:, None])
                qbk_sb = singles.tile([P, 1], F32, tag="qbk")
                nc.sync.dma_start(qbk_sb[:], qbk.ap()[:, None])
                qbv_sb = singles.tile([P, P], F32, tag="qbv")
                qbv_b = bass.AP(tensor=qbv, offset=0, ap=[[0, P], [1, P]])
                nc.gpsimd.dma_start(out=qbv_sb[:], in_=qbv_b)

            QT = [singles.tile([P, N], BF16, tag=f"QT{b}", name=f"QT{b}") for b in range(B)]
            KT = [singles.tile([P, N], BF16, tag=f"KT{b}", name=f"KT{b}") for b in range(B)]
            V0 = [singles.tile([P, NKT, HD + 1], BF16, tag=f"V0{b}", name=f"V0{b}") for b in range(B)]
            V1 = [singles.tile([P, NKT, HD + 1], BF16, tag=f"V1{b}", name=f"V1{b}") for b in range(B)]
            OT = [singles.tile([P, N], BF16, tag=f"OT{b}", name=f"OT{b}") for b in range(B)]
            for b in range(B):
                nc.vector.memset(V0[b][:, :, HD:HD + 1], 1.0)
                nc.vector.memset(V1[b][:, :, HD:HD + 1], 1.0)

            def phase_ln(b):
                """LayerNorm for batch b: x tiles -> xn tiles -> DRAM."""
                for t in range(NKT):
                    r = b * N + t * P
                    xt = pa_x.tile([P, D], BF16, tag="xt")
                    nc.sync.dma_start(xt[:], x.ap()[r:r + P, :])
                    stats = pas.tile([P, 2, 6], F32, tag="stats")
                    xt3 = xt[:].rearrange("p (s f) -> p s f", s=2)
                    nc.vector.bn_stats(stats[:, 0, :], xt3[:, 0, :])
                    nc.vector.bn_stats(stats[:, 1, :], xt3[:, 1, :])
                    mv = pas.tile([P, 2], F32, tag="mv")
                    nc.vector.bn_aggr(mv[:], stats[:])
                    std = pas.tile([P, 1], F32, tag="std")
                    nc.scalar.activation(std[:], mv[:, 1:2], AF.Sqrt, bias=eps_sb[:])
                    rs = pas.tile([P, 1], F32, tag="rs")
                    nc.vector.reciprocal(rs[:], std[:])
                    xn_t = pa_xn.tile([P, D], BF16, tag="xn")
                    nc.vector.tensor_scalar(
                        xn_t[:], xt[:], mv[:, 0:1], rs[:],
                        op0=ALU.subtract, op1=ALU.mult)
                    nc.sync.dma_start(xn_dram.ap()[r:r + P, :], xn_t[:])

            def phase_qkv(b):
                """Transpose xn back and compute Q^T/K^T/V for batch b."""
                xnT = paxnt.tile([P, DC, N], BF16, tag="xnT")
                for c in range(DC):
                    nc.sync.dma_start_transpose(
                        xnT[:, c, :],
                        xn_dram.ap()[b * N:(b + 1) * N, c * P:(c + 1) * P])
                # QKV per 512-token group
                for g in range(NGRP):
                    gsl = slice(g * GRP, (g + 1) * GRP)
                    psqk = psS.tile([P, 2, GRP], F32, tag="psS",
                                    name=f"psqk{b}_{g}")
                    for c in range(DC):
                        nc.tensor.matmul(psqk[:, 0, :], wq_sb[:, c, :],
                                         xnT[:, c, gsl],
                                         start=(c == 0), stop=(c == DC - 1))
                    for c in range(DC):
                        nc.tensor.matmul(psqk[:, 1, :], wk_sb[:, c, :],
                                         xnT[:, c, gsl],
                                         start=(c == 0), stop=(c == DC - 1))
                    if use_qkv_bias:
                        nc.vector.tensor_scalar(
                            QT[b][:, gsl], psqk[:, 0, :], qbq_sb[:], SCALE,
                            op0=ALU.add, op1=ALU.mult)
                        nc.vector.tensor_scalar_add(KT[b][:, gsl], psqk[:, 1, :],
                                                    qbk_sb[:])
                    else:
                        nc.scalar.mul(QT[b][:, gsl], psqk[:, 0, :], SCALE)
                        nc.scalar.copy(KT[b][:, gsl], psqk[:, 1, :])
                    # V for the 4 token tiles of this group, packed into bank 0
                    psv = psS.tile([P, 2, GRP], F32, tag="psS",
                                   name=f"psv{b}_{g}")
                    for t in range(GRP // P):
                        tok = slice((g * 4 + t) * P, (g * 4 + t + 1) * P)
                        for c in range(DC):
                            nc.tensor.matmul(psv[:, 0, t * P:(t + 1) * P],
                                             xnT[:, c, tok], wv_sb[:, c, :],
                                             start=(c == 0), stop=(c == DC - 1))
                    psv4 = psv[:, 0, :].rearrange("p (t v) -> p t v", t=4)
                    ksl4 = slice(g * 4, (g + 1) * 4)
                    if use_qkv_bias:
                        bv0 = qbv_sb[:, 0:HD]
                        bv1 = qbv_sb[:, HD:P]
                        nc.vector.tensor_tensor(
                            V0[b][:, ksl4, 0:HD], psv4[:, :, 0:HD],
                            bv0[:, None, :].to_broadcast((P, 4, HD)), ALU.add)
                        nc.vector.tensor_tensor(
                            V1[b][:, ksl4, 0:HD], psv4[:, :, HD:P],
                            bv1[:, None, :].to_broadcast((P, 4, HD)), ALU.add)
                    else:
                        nc.scalar.copy(V0[b][:, ksl4, 0:HD], psv4[:, :, 0:HD])
                        nc.scalar.copy(V1[b][:, ksl4, 0:HD], psv4[:, :, HD:P])

            def phase_b_qb(qb):
                """Attention for one q block across all batches; the 16 ebt
                tiles for this q block are loaded once and reused 4x."""
                qsl = slice(qb * QB, (qb + 1) * QB)
                ebts = []
                for kt in range(NKT):
                    ebt2 = pbe.tile([P, HPC, QB], BF16, tag="ebt",
                                    name=f"ebt_{qb}_{kt}")
                    nc.sync.dma_start(
                        ebt2[:],
                        ebt.ap()[:, kt, qb].rearrange("h p q -> p h q"))
                    ebts.append(ebt2)
                for b in range(B):
                    pso0 = psOY.tile([HD + 1, QB], F32, tag="po",
                                     name=f"psO0_{b}_{qb}")
                    pso1 = psOY.tile([HD + 1, QB], F32, tag="po",
                                     name=f"psO1_{b}_{qb}")
                    for kt in range(NKT):
                        ksl = slice(kt * P, (kt + 1) * P)
                        pss = psS.tile([P, 2, QB], F32, tag="psS",
                                       name=f"psS{b}_{qb}_{kt}")
                        # both heads' score matmuls: disjoint row groups (0/64)
                        nc.tensor.matmul(pss[:, 0, :], KT[b][0:HD, ksl],
                                         QT[b][0:HD, qsl], start=True, stop=True)
                        nc.tensor.matmul(pss[:, 1, :], KT[b][HD:P, ksl],
                                         QT[b][HD:P, qsl], start=True, stop=True)
                        p0 = pbp.tile([P, HPC, QB], BF16, tag="p0")
                        nc.scalar.activation(p0[:], pss[:], AF.Exp)
                        nc.vector.tensor_tensor(p0[:], p0[:], ebts[kt][:],
                                                ALU.mult)
                        nc.tensor.matmul(pso0[:], V0[b][:, kt, :], p0[:, 0, :],
                                         start=(kt == 0), stop=(kt == NKT - 1))
                        nc.tensor.matmul(pso1[:], V1[b][:, kt, :], p0[:, 1, :],
                                         start=(kt == 0), stop=(kt == NKT - 1))
                    for h, pso in ((0, pso0), (1, pso1)):
                        hsl = slice(h * HD, (h + 1) * HD)
                        rec = pbs.tile([1, QB], F32, tag="rec")
                        nc.vector.reciprocal(rec[:], pso[HD:HD + 1, :])
                        rslot = rec_dram.ap()[b, qb, h:h + 1, :]
                        nc.sync.dma_start(rslot, rec[0:1, :])
                        rb = pbs.tile([HD, QB], F32, tag="rb")
                        rec_b = bass.AP(tensor=rec_dram,
                                        offset=rslot.offset,
                                        ap=[[0, HD], [1, QB]])
                        nc.sync.dma_start(out=rb[:], in_=rec_b)
                        nc.vector.tensor_tensor(OT[b][hsl, qsl], pso[0:HD, :],
                                                rb[:], ALU.mult)

            def phase_c(b):
                """Partial out-projection for batch b."""
                for t in range(NKT):
                    tsl = slice(t * P, (t + 1) * P)
                    yt = pcy.tile([P, D], BF16, tag="yt")
                    for nb in range(D // 512):
                        nsl = slice(nb * 512, (nb + 1) * 512)
                        psy = psOY.tile([P, 512], F32, tag="po")
                        nc.tensor.matmul(psy[:], OT[b][:, tsl], wo_sb[:, nsl],
                                         start=True, stop=True)
                        if (t + nb) % 2 == 0:
                            nc.scalar.copy(yt[:, nsl], psy[:])
                        else:
                            nc.vector.tensor_copy(yt[:, nsl], psy[:])
                    nc.sync.dma_start(
                        y.ap()[b * N + t * P:b * N + (t + 1) * P, :], yt[:])

            # A section first (LN+QKV for all batches; Sqrt table stays
            # resident until the exps begin), then attention qb-outer with
            # the ebt tiles cached across batches, then the out-projections.
            for b in range(B):
                phase_ln(b)
                phase_qkv(b)
            for qb in range(NQB):
                phase_b_qb(qb)
            for b in range(B):
                phase_c(b)

    _split_waits(nc)
    return nc


_GRAPH_CACHE = {}


def _get_graph(use_qkv_bias):
    if use_qkv_bias not in _GRAPH_CACHE:
        _GRAPH_CACHE[use_qkv_bias] = _build_graph(use_qkv_bias)
    return _GRAPH_CACHE[use_qkv_bias]


def kernel(x, relative_position_bias, w_qkv, w_out, b_out, ln_gamma, ln_beta,
           _run_kwargs=None):
    x = np.asarray(x, dtype=np.float32)
    bias = np.asarray(relative_position_bias, dtype=np.float32)
    w_qkv = np.asarray(w_qkv, dtype=np.float32)
    w_out = np.asarray(w_out, dtype=np.float32)
    b_out = np.asarray(b_out, dtype=np.float32)
    ln_gamma = np.asarray(ln_gamma, dtype=np.float32)
    ln_beta = np.asarray(ln_beta, dtype=np.float32)

    # fold LN affine into the QKV projection
    w = w_qkv * ln_gamma[:, None]                       # [D, 3D]
    qkv_bias = ln_beta @ w_qkv                          # [3D]
    use_qkv_bias = bool(np.any(qkv_bias != 0.0))

    x_bf = np.ascontiguousarray(x.reshape(TOK, D)).astype(NPBF16)
    eb = np.exp(bias)                                   # [16, N, N]

    in_maps = []
    for c in range(NCORES):
        h0 = HPC * c
        csl = slice(h0 * HD, (h0 + HPC) * HD)
        m = {
            "x": x_bf,
            "wq": np.ascontiguousarray(w[:, csl]).astype(NPBF16),
            "wk": np.ascontiguousarray(w[:, D + h0 * HD:D + (h0 + HPC) * HD]).astype(NPBF16),
            "wv": np.ascontiguousarray(w[:, 2 * D + h0 * HD:2 * D + (h0 + HPC) * HD]).astype(NPBF16),
            "wo": np.ascontiguousarray(w_out[csl, :]).astype(NPBF16),
            # [h, kt, qb, p(k-within-chunk), q] with each [p, q] tile contiguous
            "ebt": np.ascontiguousarray(
                eb[h0:h0 + HPC].transpose(0, 2, 1)          # [h, k, q]
                .reshape(HPC, NKT, P, NQB, QB)
                .transpose(0, 1, 3, 2, 4)).astype(NPBF16),
        }
        if use_qkv_bias:
            m["qbq"] = np.ascontiguousarray(qkv_bias[csl])
            m["qbk"] = np.ascontiguousarray(qkv_bias[D + h0 * HD:D + (h0 + HPC) * HD])
            m["qbv"] = np.ascontiguousarray(qkv_bias[2 * D + h0 * HD:2 * D + (h0 + HPC) * HD])
        in_maps.append(m)

    nc = _get_graph(use_qkv_bias)
    kwargs = dict(_run_kwargs or {})
    res = run_bass_kernel_spmd(nc, in_maps, core_ids=list(range(NCORES)), **kwargs)

    acc = np.zeros((TOK, D), dtype=np.float32)
    for c in range(NCORES):
        acc += np.asarray(res.results[c]["out"], dtype=np.float32)
    out = acc + b_out[None, :]
    if _run_kwargs is not None:
        kernel.last_result = res
    return out.reshape(B, N, D).astype(np.float32)

